# revision 10
# baseline (speedup 1.0000x reference)
"""Causal self-attention on 8 TRN2 NeuronCores — hybrid fp8 redesign.

Sharding: data-parallel over batch (2) x tensor-parallel over heads (4 heads
per core). Core c handles batch c//4, heads 4*(c%4)..4*(c%4)+3 — i.e. columns
[256*g, 256*(g+1)) of wq/wk/wv and rows [256*g, 256*(g+1)) of wo. Each core
returns a partial output [2048, 1024]; the host sums the 4 partials of each
batch and adds the (bv @ wo + bo) correction (exact because softmax rows sum
to 1).

Per-core kernel (Tile framework, fully unrolled, f32 psum):
  1. Host pre-transposes x; fp8 xT chunks DMA into SBUF (bf16 xT only for
     token-block 0). qT/kT projected fp8-DoubleRow (chunk pairs) with bias +
     scale fused into the psum->sbuf move. v: token-block 0 via bf16 matmuls
     (precision: early queries attend to few keys, so no error averaging),
     blocks 1-3 via fp8 DoubleRow; v lands in v_aug (ones column per head so
     the AV matmul also produces softmax denominators) — bf16 copy for
     chunks 0-3 (block-0 AV) plus fp8 copy for all chunks.
  2. Scores per (head-pair, 512-wide i-block) kept TRANSPOSED ([l-chunk=128,
     i=512]) in bf16 qt/kt; chunks above the diagonal are skipped; diagonal
     chunks compute only the live column range and are masked by ACCUMULATING
     a strictly-triangular -60 tile onto the scores via a tiny PE matmul
     (exp underflows to 0).
  3. exp: i-block 0 -> bf16 per-chunk tiles; i-blocks 1-3 -> fp8e4m3 PAIR
     tiles (two key-chunks side by side). AV with the query dim on output
     partitions: i-block 0 uses bf16 single-chunk matmuls; i-blocks 1-3 use
     fp8 DoubleRow over chunk pairs (half the instructions, quarter the PE
     cycles) + an fp8 single for odd window lengths. The hybrid keeps the
     max-error element (early query rows) at bf16 precision. Denominator
     lands as a per-partition column -> reciprocal + one broadcast
     tensor_tensor multiply per (i-block, head) writes normalized ao.
  4. Attention output tiles [128 t, 256 j] are PE-transposed back to [j, t]
     (bf16) for the bf16 out-projection; y copies ride Pool/ACT (DVE is
     loaded), trp copies ride Pool.
  5. Schedule: software-pipelined — next block's projections interleave with
     the current block's ACT(exp)-paced attention chunks, weighted by PE
     starvation; deferred out-projections park in the last block. PSUM =
     2x[128,1024] scores + 2x[128,260] AV + 2x[128,512] fillers = 8 banks.
"""

import sys

import numpy as np

if "/opt/trn_rl_repo" not in sys.path:
    sys.path.insert(0, "/opt/trn_rl_repo")

import concourse.mybir as mybir
import concourse.tile as tile
from concourse import bacc
from concourse.bass_utils import run_bass_kernel_spmd

# Problem shapes (hardcoded per contract)
B, S, D = 2, 2048, 1024
H, DH = 16, 64
NCORES = 8
GROUPS = 4                  # tensor-parallel groups per batch
HL = H // GROUPS            # 4 local heads
JC = HL * DH                # 256 local head columns
T = S                       # tokens per core (one batch element)

P = 128                     # partitions
TS = 512                    # token block (projection granularity)
NTB = T // TS               # 4 token blocks
NDC = D // P                # 8 contraction chunks
IB = 512                    # attention i-block (query positions)
LCH = P                     # attention l-chunk (key positions)
VA = DH + 1                 # v_aug columns per head (ones column appended)
NIT = IB // P               # 4 i-tiles per block

FP = mybir.dt.float32
BF = mybir.dt.bfloat16
F8 = mybir.dt.float8e4

_CACHE = {}


def build_nc():
    nc = bacc.Bacc("TRN2", target_bir_lowering=False, debug=False)

    import os
    xt_d = nc.dram_tensor("xt", [D, TS], BF, kind="ExternalInput")  # block 0 only
    xt8_d = nc.dram_tensor("xt8", [D, T], F8, kind="ExternalInput")
    wq = nc.dram_tensor("wq", [D, JC], F8, kind="ExternalInput")
    wk = nc.dram_tensor("wk", [D, JC], F8, kind="ExternalInput")
    wv = nc.dram_tensor("wv", [D, JC], F8, kind="ExternalInput")
    wv_bf = nc.dram_tensor("wv_bf", [D, JC], BF, kind="ExternalInput")
    wo = nc.dram_tensor("wo", [JC, D], BF, kind="ExternalInput")
    bq = nc.dram_tensor("bq", [JC, 1], FP, kind="ExternalInput")
    bk = nc.dram_tensor("bk", [JC, 1], FP, kind="ExternalInput")
    y = nc.dram_tensor("y", [T, D], BF, kind="ExternalOutput")
    # schedule-tuning knobs (env for experiments)
    exp_dve_mod = int(os.environ.get("K_EXP_DVE_MOD", "0"))  # 0 = off
    yact = os.environ.get("K_YACT", "0") == "1"
    n_warm = int(os.environ.get("K_WARM", "56"))
    w_full = float(os.environ.get("K_W_FULL", "3.0"))
    w_diag = float(os.environ.get("K_W_DIAG", "1.0"))

    with tile.TileContext(nc) as tc:
        import contextlib

        with contextlib.ExitStack() as ctx:
            singles = ctx.enter_context(tc.tile_pool(name="singles", bufs=1))
            xt_pool = ctx.enter_context(tc.tile_pool(name="xt", bufs=2))
            # all of an i-block pair's exp tiles stay live until its AV pass
            exp_pool = ctx.enter_context(tc.tile_pool(name="exp", bufs=32))
            nrm_pool = ctx.enter_context(tc.tile_pool(name="nrm", bufs=4))
            ysb_pool = ctx.enter_context(tc.tile_pool(name="ysb", bufs=4))
            # PSUM: tag "big" 2x[128,1024] f32 (qT/kT proj pairs, then score
            # pairs), "av" 2x[128,260] f32 (AV + denominators, one per head),
            # "fil" 2x[128,512] (proj, aoT transposes, out-proj) = 8 banks.
            ps = ctx.enter_context(tc.tile_pool(name="ps", bufs=2, space="PSUM"))

            def load_block(tb, halves=1):
                # fp8 xT always; bf16 xT only for block 0 (bf16 v-projection)
                xt8 = xt_pool.tile([P, NDC * TS], F8, tag="xt8", name=f"xt8_{tb}")
                src8 = xt8_d[:, :].rearrange("(c p) t -> p c t", p=P)[
                    :, :, TS * tb:TS * (tb + 1)]
                h = NDC // halves
                for k in range(halves):
                    nc.sync.dma_start(
                        out=xt8.rearrange("p (c t) -> p c t", t=TS)[
                            :, h * k:h * (k + 1), :],
                        in_=src8[:, h * k:h * (k + 1), :],
                    )
                xt = None
                if tb == 0:
                    xt = xt_pool.tile([P, NDC * TS], BF, tag="xt", name=f"xt{tb}")
                    src = xt_d[:, :].rearrange("(c p) t -> p c t", p=P)
                    for k in range(halves):
                        nc.sync.dma_start(
                            out=xt.rearrange("p (c t) -> p c t", t=TS)[
                                :, h * k:h * (k + 1), :],
                            in_=src[:, h * k:h * (k + 1), :],
                        )
                return xt, xt8

            # ---- DMA order = need order: wq/bq + first xt half unblock the
            # first projection ~2.5us in; wo is not needed until ~40us ----
            wq_sb = singles.tile([P, NDC * JC], F8, tag="wq")  # chunk c at [JC*c, JC*(c+1))
            wk_sb = singles.tile([P, NDC * JC], F8, tag="wk")
            wv_sb = singles.tile([P, NDC * JC], F8, tag="wv")
            wvb_sb = singles.tile([P, NDC * JC], BF, tag="wvb")
            bq_sb = singles.tile([P, 2], FP, tag="bq")
            bk_sb = singles.tile([P, 2], FP, tag="bk")
            wo_sb = singles.tile([P, 2 * D], BF, tag="wo")      # j-chunk jc at [D*jc, ...)
            nc.sync.dma_start(
                out=wq_sb.rearrange("p (c j) -> p c j", j=JC),
                in_=wq[:, :].rearrange("(c p) j -> p c j", p=P),
            )
            nc.sync.dma_start(
                out=bq_sb, in_=bq[:, :].rearrange("(j p) one -> p (j one)", p=P),
            )
            nc.sync.dma_start(
                out=wk_sb.rearrange("p (c j) -> p c j", j=JC),
                in_=wk[:, :].rearrange("(c p) j -> p c j", p=P),
            )
            nc.sync.dma_start(
                out=bk_sb, in_=bk[:, :].rearrange("(j p) one -> p (j one)", p=P),
            )
            xt0 = load_block(0, halves=2)
            nc.sync.dma_start(
                out=wvb_sb.rearrange("p (c j) -> p c j", j=JC),
                in_=wv_bf[:, :].rearrange("(c p) j -> p c j", p=P),
            )
            nc.sync.dma_start(
                out=wv_sb.rearrange("p (c j) -> p c j", j=JC),
                in_=wv[:, :].rearrange("(c p) j -> p c j", p=P),
            )
            nc.sync.dma_start(
                out=wo_sb.rearrange("p (c d) -> p c d", d=D),
                in_=wo[:, :].rearrange("(c p) d -> p c d", p=P),
            )

            # bf16 identity for the ao PE transposes
            ident = singles.tile([P, P], BF)
            nc.vector.memset(ident, 0.0)
            nc.gpsimd.affine_select(
                out=ident, in_=ident, compare_op=mybir.AluOpType.not_equal,
                fill=1.0, base=0, channel_multiplier=1, pattern=[[-1, P]],
            )
            # strictly-upper-triangular -BIG tile (transposed; stationary of
            # the causal-mask matmul: TRIT.T @ I adds -BIG where key > query)
            trit = singles.tile([P, P], BF)
            nc.vector.memset(trit, 0.0)
            nc.gpsimd.affine_select(
                out=trit, in_=trit, compare_op=mybir.AluOpType.is_ge,
                fill=-60.0, base=0, channel_multiplier=1, pattern=[[-1, P]],
            )

            # persistent activations
            qt_sb = [singles.tile([P, T], BF, tag=f"qt{j}", name=f"qt_sb{j}") for j in range(2)]
            kt_sb = [singles.tile([P, T], BF, tag=f"kt{j}", name=f"kt_sb{j}") for j in range(2)]
            # attention out, natural layout: tile tt holds [128 t, 4 heads x 64]
            ao_nat = singles.tile([P, (T // P) * HL * DH], BF, tag="aon")
            # attention out transposed: j-chunk jc at cols [T*jc, T*(jc+1))
            aoT_sb = singles.tile([P, 2 * T], BF, tag="aot")
            # v_aug fp8 (all chunks): l-chunk lc at [VA*HL*lc, ...), head h at
            # offset VA*h, ones at +DH.  bf16 copy of chunks 0-3 for block-0 AV.
            n_lch = T // LCH
            vaug = singles.tile([P, n_lch * HL * VA], F8, tag="vaug")
            vaug_g = vaug.rearrange("p (c v) -> p c v", v=VA)
            vaug_p = vaug.rearrange("p (c h v) -> p c h v", h=HL, v=VA)
            nc.vector.memset(vaug_g[:, :, DH], 1.0)   # ones columns
            vaugb = singles.tile([P, NIT * HL * VA], BF, tag="vaugb")
            vaugb_g = vaugb.rearrange("p (c v) -> p c v", v=VA)
            nc.vector.memset(vaugb_g[:, :, DH], 1.0)

            # PE warm-up: dummy matmuls during the initial DMA wait get the
            # clock to full rate before the real work arrives.
            warm = ps.tile([P, P], FP, tag="fil", name="warm")
            junk = singles.tile([P, P], BF, tag="junk")
            nc.vector.memset(junk, 0.0)
            for _ in range(8):
                nc.tensor.matmul(warm, junk, junk, start=True, stop=True)
            for _ in range(n_warm - 8):
                nc.tensor.matmul(warm, ident, ident, start=True, stop=True)

            # ---------- emission units (software-pipelined schedule) ----------
            def proj_units(tb, xt, xt8):
                """Single-bank filler units: q/k transposed per j-tile, v in
                natural [token, head-col] layout straight into v_aug."""
                units = []
                xt8_g = xt8.rearrange("p (c t) -> p c t", t=TS)

                def qk_mm(acc, w_sb, j, c, start, stop):
                    # fp8 DoubleRow: chunk PAIR (2c, 2c+1) packed as k-tiles
                    w_g = w_sb.rearrange("p (c j) -> p c j", j=JC)
                    nc.tensor.matmul(
                        acc,
                        w_g[:, 2 * c:2 * c + 2, P * j:P * (j + 1)],
                        xt8_g[:, 2 * c:2 * c + 2, :],
                        start=start, stop=stop,
                        perf_mode=mybir.MatmulPerfMode.DoubleRow,
                    )

                def make_qk(which, w_sb, out_sb, j):
                    box = [None]
                    nqk = NDC // 2   # chunk-pairs

                    def emit_lo():
                        box[0] = ps.tile([P, TS], FP, tag="fil", name=f"{which}p{tb}_{j}")
                        for c in range(nqk // 2):
                            qk_mm(box[0], w_sb, j, c, start=(c == 0), stop=False)

                    def emit_hi():
                        acc = box[0]
                        for c in range(nqk // 2, nqk):
                            qk_mm(acc, w_sb, j, c, start=False, stop=(c == nqk - 1))
                        if which == "qt":
                            nc.vector.tensor_scalar(
                                out=out_sb[j][:, TS * tb:TS * (tb + 1)], in0=acc,
                                scalar1=0.125, scalar2=bq_sb[:, j:j + 1],
                                op0=mybir.AluOpType.mult, op1=mybir.AluOpType.add,
                            )
                        else:
                            nc.vector.tensor_scalar(
                                out=out_sb[j][:, TS * tb:TS * (tb + 1)], in0=acc,
                                scalar1=bk_sb[:, j:j + 1], scalar2=None,
                                op0=mybir.AluOpType.add,
                            )
                    return [emit_lo, emit_hi]

                def make_v(s):
                    box = [None]

                    def make_w(w):
                        def emit():
                            # natural [t, j] layout (xT chunk is the stationary);
                            # sequential accumulation groups per bank half
                            if w == 0:
                                box[0] = ps.tile([P, TS], FP, tag="fil", name=f"vp{tb}_{s}")
                            acc = box[0]
                            ts_ = 2 * s + w
                            if tb == 0:
                                # bf16: block-0 queries need full-precision v
                                for c in range(NDC):
                                    nc.tensor.matmul(
                                        acc[:, JC * w:JC * (w + 1)],
                                        xt[:, TS * c + P * ts_:TS * c + P * (ts_ + 1)],
                                        wvb_sb[:, JC * c:JC * (c + 1)],
                                        start=(c == 0), stop=(c == NDC - 1),
                                    )
                            else:
                                # fp8 DoubleRow over chunk pairs
                                wv_g = wv_sb.rearrange("p (c j) -> p c j", j=JC)
                                for c in range(NDC // 2):
                                    nc.tensor.matmul(
                                        acc[:, JC * w:JC * (w + 1)],
                                        xt8_g[:, 2 * c:2 * c + 2,
                                              P * ts_:P * (ts_ + 1)],
                                        wv_g[:, 2 * c:2 * c + 2, :],
                                        start=(c == 0), stop=(c == NDC // 2 - 1),
                                        perf_mode=mybir.MatmulPerfMode.DoubleRow,
                                    )
                            lc = 4 * tb + ts_
                            if tb == 0:
                                # bf16 copy for block-0 AV; fp8 vaug derived
                                # SBUF->SBUF on gpsimd (no PSUM port there)
                                nc.vector.tensor_copy(
                                    out=vaugb_g[:, HL * lc:HL * (lc + 1), 0:DH],
                                    in_=acc[:, JC * w:JC * (w + 1)].rearrange(
                                        "p (h d) -> p h d", d=DH
                                    ),
                                )
                                nc.gpsimd.tensor_copy(
                                    out=vaug_g[:, HL * lc:HL * (lc + 1), 0:DH],
                                    in_=vaugb_g[:, HL * lc:HL * (lc + 1), 0:DH],
                                )
                            else:
                                nc.vector.tensor_copy(
                                    out=vaug_g[:, HL * lc:HL * (lc + 1), 0:DH],
                                    in_=acc[:, JC * w:JC * (w + 1)].rearrange(
                                        "p (h d) -> p h d", d=DH
                                    ),
                                )
                        return emit
                    return [make_w(0), make_w(1)]

                for j in range(2):
                    units.extend(make_qk("qt", wq_sb, qt_sb, j))
                    units.extend(make_qk("kt", wk_sb, kt_sb, j))
                for s in range(2):
                    units.extend(make_v(s))
                return units

            def attn_units(i):
                """Returns (close_units, close_weights): full-chunk stretch is
                ACT(exp)-paced (weight ~ starvation); close stretch (diagonal
                chunks + AV windows + tails) is PE-rich."""
                nch = 4 * (i + 1)   # causal chunks
                fp8 = i >= 1        # hybrid: block 0 keeps bf16 AV
                cunits = []
                cweights = []
                for jp in range(2):          # head pair (2*jp, 2*jp+1)
                    avs = [None, None]
                    # i==0: per-chunk bf16 tiles; i>=1: fp8 pair tiles
                    exs = [None] * nch
                    ex2s = [None] * ((nch + 1) // 2)

                    def make_pair_start(i, jp, avs):
                        def emit():
                            for u in range(2):
                                avs[u] = ps.tile(
                                    [P, NIT * VA], FP, tag="av", name=f"av{i}_{2 * jp + u}"
                                )
                        return emit

                    def make_c(i, jp, exs, ex2s, c, dve_exp=False):
                        def emit():
                            # Diagonal chunks only need query columns >= 128*v
                            diag = c >= 4 * i
                            v = c - 4 * i if diag else 0
                            off = P * v
                            # both heads' scoresT for chunk c in one 2-bank tile
                            sc = ps.tile([P, 2 * IB], FP, tag="big", name=f"sc{i}_{jp}_{c}")
                            for u in range(2):
                                ro = DH * u
                                nc.tensor.matmul(
                                    sc[:, IB * u + off:IB * (u + 1)],
                                    kt_sb[jp][ro:ro + DH, LCH * c:LCH * (c + 1)],
                                    qt_sb[jp][ro:ro + DH, IB * i + off:IB * (i + 1)],
                                    start=True, stop=not diag,
                                )
                                if diag:
                                    # causal mask: add -60 where key > query
                                    nc.tensor.matmul(
                                        sc[:, IB * u + off:IB * u + off + P],
                                        trit, ident,
                                        start=False, stop=True,
                                        skip_group_check=True,
                                    )
                            sc_g = sc.rearrange("p (u n) -> p u n", u=2)
                            if not fp8:
                                ex = exp_pool.tile([P, 2 * IB], BF, tag="ex",
                                                   name=f"ex{i}_{jp}_{c}")
                                exs[c] = ex
                                ex_t = ex.rearrange("p (u n) -> p u n", u=2)[:, :, off:]
                            else:
                                if c % 2 == 0:
                                    ex2s[c // 2] = exp_pool.tile(
                                        [P, 2 * 2 * IB], F8, tag="ex",
                                        name=f"ex{i}_{jp}_{c // 2}p")
                                ex2 = ex2s[c // 2]
                                ex_t = ex2.rearrange(
                                    "p (k u n) -> p k u n", k=2, u=2
                                )[:, c % 2, :, off:]
                            if dve_exp:
                                # bit-trick exp on DVE (bf16 bits of e^x ~
                                # round(x*128/ln2 + magic)), then idle Pool
                                # converts bf16->fp8 SBUF->SBUF (gpsimd has
                                # no PSUM port and int8 DVE writes fail)
                                scr = nrm_pool.tile([P, 2 * IB], BF, tag="exs",
                                                    name=f"exs{i}_{jp}_{c}")
                                nc.vector.tensor_scalar(
                                    out=scr.bitcast(mybir.dt.int16), in0=sc,
                                    scalar1=184.66500816464, scalar2=16248.6,
                                    op0=mybir.AluOpType.mult,
                                    op1=mybir.AluOpType.add,
                                )
                                nc.gpsimd.tensor_copy(
                                    out=ex_t,
                                    in_=scr.rearrange("p (u n) -> p u n", u=2),
                                )
                            else:
                                nc.scalar.activation(
                                    out=ex_t, in_=sc_g[:, :, off:],
                                    func=mybir.ActivationFunctionType.Exp,
                                )
                        return emit

                    def make_av(i, jp, avs, exs, ex2s, u, it):
                        # one contiguous PSUM accumulation window per (head,
                        # i-tile)
                        def emit():
                            h = 2 * jp + u
                            cnt = 4 * i + it + 1
                            out = avs[u][:, VA * it:VA * (it + 1)]
                            if not fp8:
                                for c in range(cnt):
                                    nc.tensor.matmul(
                                        out,
                                        exs[c][:, IB * u + P * it:IB * u + P * (it + 1)],
                                        vaugb_g[:, HL * c + h, :],
                                        start=(c == 0), stop=(c == cnt - 1),
                                        skip_group_check=True,
                                    )
                            else:
                                npair = cnt // 2
                                odd = cnt % 2
                                for pc in range(npair):
                                    ex_pair = ex2s[pc].rearrange(
                                        "p (k u n) -> p k u n", k=2, u=2
                                    )[:, :, u, P * it:P * (it + 1)]
                                    nc.tensor.matmul(
                                        out,
                                        ex_pair,
                                        vaug_p[:, 2 * pc:2 * pc + 2, h, :],
                                        start=(pc == 0), stop=(not odd and pc == npair - 1),
                                        perf_mode=mybir.MatmulPerfMode.DoubleRow,
                                        skip_group_check=True,
                                    )
                                if odd:
                                    ex_last = ex2s[npair].rearrange(
                                        "p (k u n) -> p k u n", k=2, u=2
                                    )[:, 0, u, P * it:P * (it + 1)]
                                    nc.tensor.matmul(
                                        out,
                                        ex_last,
                                        vaug_g[:, HL * (cnt - 1) + h, :],
                                        start=(npair == 0), stop=True,
                                        skip_group_check=True,
                                    )
                        return emit

                    def make_tail(i, jp, avs, u):
                        def emit():
                            h = 2 * jp + u
                            av_g = avs[u].rearrange("p (t v) -> p t v", v=VA)
                            rc = nrm_pool.tile([P, NIT], FP, tag="rc", name=f"rc{i}_{h}")
                            nc.vector.reciprocal(out=rc, in_=av_g[:, :, DH])
                            # one broadcast multiply normalizes all 4 i-tiles
                            ao_v = ao_nat.rearrange("p (t j) -> p t j", j=JC)[
                                :, NIT * i:NIT * (i + 1),
                                P * jp + DH * u:P * jp + DH * (u + 1)]
                            rc_b = rc.rearrange("p (t o) -> p t o", o=1).to_broadcast(
                                [P, NIT, DH])
                            nc.vector.tensor_tensor(
                                out=ao_v, in0=av_g[:, :, 0:DH], in1=rc_b,
                                op=mybir.AluOpType.mult,
                            )
                        return emit

                    units, weights = cunits, cweights   # per-jp: full then close
                    for c in range(4 * i):
                        if exp_dve_mod > 0:
                            dve = i >= 2 and c % exp_dve_mod == 2 % exp_dve_mod
                        elif exp_dve_mod < 0:
                            dve = i == 3 and c % -exp_dve_mod == 2 % -exp_dve_mod
                        else:
                            dve = False
                        units.append(make_c(i, jp, exs, ex2s, c, dve_exp=dve))
                        weights.append(w_full)
                    cunits.append(make_pair_start(i, jp, avs))
                    cweights.append(0.0)
                    # AV window (u, it) completes with diagonal chunk 4i+it:
                    # emit it one diag chunk later so its tail never waits on
                    # the freshest exp
                    for v in range(NIT):
                        cunits.append(make_c(i, jp, exs, ex2s, 4 * i + v))
                        cweights.append(w_diag)
                        if v >= 1:
                            for u in range(2):
                                cunits.append(make_av(i, jp, avs, exs, ex2s, u, v - 1))
                                cweights.append(0.0)
                    for u in range(2):
                        cunits.append(make_av(i, jp, avs, exs, ex2s, u, NIT - 1))
                        cweights.append(0.0)
                    cunits.append(make_tail(i, jp, avs, 0))
                    cweights.append(0.0)
                    cunits.append(make_tail(i, jp, avs, 1))
                    cweights.append(0.0)
                return cunits, cweights

            def y_units(i):
                units = []

                def make(tt):
                    ysb_box = [None]

                    def emit_tr():
                        trp = ps.tile([P, 2 * P], BF, tag="fil", name=f"tr{tt}")
                        for jc in range(2):
                            nc.tensor.transpose(
                                trp[:, P * jc:P * (jc + 1)],
                                ao_nat[:, JC * tt + P * jc:JC * tt + P * (jc + 1)],
                                ident,
                            )
                        # one copy moves both j-chunks into aoT (jc-major)
                        nc.vector.tensor_copy(
                            out=aoT_sb.rearrange("p (j t) -> p j t", j=2)[
                                :, :, P * tt:P * (tt + 1)],
                            in_=trp.rearrange("p (j t) -> p j t", j=2),
                        )

                    def make_yp(db):
                        def emit():
                            if db == 0:
                                ysb_box[0] = ysb_pool.tile(
                                    [P, D], BF, tag="ysb", name=f"ysb{tt}"
                                )
                            ysb = ysb_box[0]
                            yps = ps.tile([P, IB], FP, tag="fil", name=f"yps{tt}_{db}")
                            for jc in range(2):
                                nc.tensor.matmul(
                                    yps,
                                    aoT_sb[:, T * jc + P * tt:P * (tt + 1) + T * jc],
                                    wo_sb[:, D * jc + IB * db:D * jc + IB * (db + 1)],
                                    start=(jc == 0), stop=(jc == 1),
                                )
                            # y copies on DVE; optionally the last block's go
                            # to ACT (idle once the final exps are done)
                            if yact and tt >= 12 and (tt + db) % 2 == 0:
                                nc.scalar.activation(
                                    out=ysb[:, IB * db:IB * (db + 1)], in_=yps,
                                    func=mybir.ActivationFunctionType.Copy,
                                )
                            else:
                                nc.vector.tensor_copy(
                                    out=ysb[:, IB * db:IB * (db + 1)], in_=yps,
                                )
                            nc.sync.dma_start(
                                out=y[P * tt:P * (tt + 1), IB * db:IB * (db + 1)],
                                in_=ysb[:, IB * db:IB * (db + 1)],
                            )
                        return emit
                    return [emit_tr, make_yp(0), make_yp(1)]

                for tt in range(NIT * i, NIT * (i + 1)):
                    units.extend(make(tt))
                return units

            def interleave(main, fillers, weights=None):
                """Emit `main` units with `fillers` spread between them,
                proportionally to per-unit `weights` (PE-starvation demand)."""
                if not main:
                    for f in fillers:
                        f()
                    return
                if weights is None:
                    weights = [1.0] * len(main)
                tot = sum(weights) or 1.0
                nf = len(fillers)
                fi = 0
                acc = 0.0
                for m, w in zip(main, weights):
                    m()
                    acc += w
                    want = int(round(acc / tot * nf))
                    while fi < want:
                        fillers[fi]()
                        fi += 1
                while fi < nf:
                    fillers[fi]()
                    fi += 1

            # ---------- pipelined schedule ----------
            # NOTE: Tile is a *tracing* scheduler — emission order defines the
            # dataflow. Every consumer must be emitted after its producer.
            for u in proj_units(0, *xt0):
                u()

            # blocks 0-2: attention + next block's load/projections
            for tb in range(3):
                cu, cw = attn_units(tb)
                nxt = load_block(tb + 1)
                interleave(cu, proj_units(tb + 1, *nxt), cw)
            # the last block's attention is ACT(exp)-bound with PE slack and
            # no next-block projections: park ALL deferred out-projection
            # blocks here as PE filler
            cu, cw = attn_units(3)
            interleave(cu, y_units(0) + y_units(1) + y_units(2), cw)
            for u in y_units(NTB - 1):
                u()

    nc.compile()
    return nc


def get_nc():
    if "nc" not in _CACHE:
        _CACHE["nc"] = build_nc()
    return _CACHE["nc"]


def kernel(x, wq, bq, wk, bk, wv, bv, wo, bo):
    import ml_dtypes
    BF_NP = ml_dtypes.bfloat16
    F8_NP = ml_dtypes.float8_e4m3

    x = np.asarray(x, dtype=np.float32)
    wq = np.asarray(wq, dtype=np.float32)
    wk = np.asarray(wk, dtype=np.float32)
    wv = np.asarray(wv, dtype=np.float32)
    wo = np.asarray(wo, dtype=np.float32)
    bq = np.asarray(bq, dtype=np.float32)
    bk = np.asarray(bk, dtype=np.float32)
    bv = np.asarray(bv, dtype=np.float32)
    bo = np.asarray(bo, dtype=np.float32)

    nc = get_nc()
    xt_f32 = [np.ascontiguousarray(x[b].T) for b in range(B)]
    xt_by_batch = [np.ascontiguousarray(xb[:, :TS]).astype(BF_NP) for xb in xt_f32]
    xt8_by_batch = [xb.astype(F8_NP) for xb in xt_f32]
    wq_c = wq.astype(F8_NP)
    wk_c = wk.astype(F8_NP)
    wv_f8 = wv.astype(F8_NP)
    wv_bf = wv.astype(BF_NP)
    wo_bf = wo.astype(BF_NP)

    in_maps = []
    for core in range(NCORES):
        b, g = divmod(core, GROUPS)
        cs = slice(JC * g, JC * (g + 1))
        im = {
            "xt": xt_by_batch[b],
            "xt8": xt8_by_batch[b],
            "wq": np.ascontiguousarray(wq_c[:, cs]),
            "wk": np.ascontiguousarray(wk_c[:, cs]),
            "wv": np.ascontiguousarray(wv_f8[:, cs]),
            "wv_bf": np.ascontiguousarray(wv_bf[:, cs]),
            "wo": np.ascontiguousarray(wo_bf[cs, :]),
            "bq": np.ascontiguousarray(bq[cs].reshape(JC, 1)),
            "bk": np.ascontiguousarray(bk[cs].reshape(JC, 1)),
        }
        in_maps.append(im)
    res = run_bass_kernel_spmd(nc, in_maps, list(range(NCORES)))
    _CACHE["last_results"] = res

    out = np.zeros((B, S, D), np.float32)
    for core in range(NCORES):
        out[core // GROUPS] += res.results[core]["y"]
    # bv and bo never pass through softmax nonlinearity: rows of attn sum to 1,
    # so (v + bv) contributes exactly bv @ wo to every output row.
    out += (bv @ wo + bo)[None, None, :]
    return out


# revision 33
# speedup vs baseline: 1.0231x; 1.0231x over previous
"""Causal self-attention on 8 TRN2 NeuronCores — hybrid fp8 redesign.

Sharding: data-parallel over batch (2) x tensor-parallel over heads (4 heads
per core). Core c handles batch c//4, heads 4*(c%4)..4*(c%4)+3 — i.e. columns
[256*g, 256*(g+1)) of wq/wk/wv and rows [256*g, 256*(g+1)) of wo. Each core
returns a partial output [2048, 1024]; the host sums the 4 partials of each
batch and adds the (bv @ wo + bo) correction (exact because softmax rows sum
to 1).

Per-core kernel (Tile framework, fully unrolled, f32 psum):
  1. Host pre-transposes x; fp8 xT chunks DMA into SBUF (bf16 xT only for
     token-block 0). qT/kT projected fp8-DoubleRow (chunk pairs) with bias +
     scale fused into the psum->sbuf move. v: token-block 0 via bf16 matmuls
     (precision: early queries attend to few keys, so no error averaging),
     blocks 1-3 via fp8 DoubleRow; v lands in v_aug (ones column per head so
     the AV matmul also produces softmax denominators) — bf16 copy for
     chunks 0-3 (block-0 AV) plus fp8 copy for all chunks.
  2. Scores per (head-pair, 512-wide i-block) kept TRANSPOSED ([l-chunk=128,
     i=512]) in bf16 qt/kt; chunks above the diagonal are skipped; diagonal
     chunks compute only the live column range and are masked by ACCUMULATING
     a strictly-triangular -60 tile onto the scores via a tiny PE matmul
     (exp underflows to 0).
  3. exp: i-block 0 -> bf16 per-chunk tiles; i-blocks 1-3 -> fp8e4m3 PAIR
     tiles (two key-chunks side by side). AV with the query dim on output
     partitions: i-block 0 uses bf16 single-chunk matmuls; i-blocks 1-3 use
     fp8 DoubleRow over chunk pairs (half the instructions, quarter the PE
     cycles) + an fp8 single for odd window lengths. The hybrid keeps the
     max-error element (early query rows) at bf16 precision. Denominator
     lands as a per-partition column -> reciprocal + one broadcast
     tensor_tensor multiply per (i-block, head) writes normalized ao.
  4. Attention output tiles [128 t, 256 j] are PE-transposed back to [j, t]
     (bf16) for the bf16 out-projection; y copies ride Pool/ACT (DVE is
     loaded), trp copies ride Pool.
  5. Schedule: software-pipelined — next block's projections interleave with
     the current block's ACT(exp)-paced attention chunks, weighted by PE
     starvation; deferred out-projections park in the last block. PSUM =
     2x[128,1024] scores + 2x[128,260] AV + 2x[128,512] fillers = 8 banks.
"""

import sys

import numpy as np

if "/opt/trn_rl_repo" not in sys.path:
    sys.path.insert(0, "/opt/trn_rl_repo")

import concourse.mybir as mybir
import concourse.tile as tile
from concourse import bacc
from concourse.bass_utils import run_bass_kernel_spmd

# Problem shapes (hardcoded per contract)
B, S, D = 2, 2048, 1024
H, DH = 16, 64
NCORES = 8
GROUPS = 4                  # tensor-parallel groups per batch
HL = H // GROUPS            # 4 local heads
JC = HL * DH                # 256 local head columns
T = S                       # tokens per core (one batch element)

P = 128                     # partitions
TS = 512                    # token block (projection granularity)
NTB = T // TS               # 4 token blocks
NDC = D // P                # 8 contraction chunks
IB = 512                    # attention i-block (query positions)
LCH = P                     # attention l-chunk (key positions)
VA = DH + 1                 # v_aug columns per head (ones column appended)
NIT = IB // P               # 4 i-tiles per block

FP = mybir.dt.float32
BF = mybir.dt.bfloat16
F8 = mybir.dt.float8e4

_CACHE = {}


def build_nc():
    nc = bacc.Bacc("TRN2", target_bir_lowering=False, debug=False)

    import os
    xt_d = nc.dram_tensor("xt", [D, TS], BF, kind="ExternalInput")  # block 0 only
    xt8_d = nc.dram_tensor("xt8", [D, T], F8, kind="ExternalInput")
    wq = nc.dram_tensor("wq", [D, JC], F8, kind="ExternalInput")
    wk = nc.dram_tensor("wk", [D, JC], F8, kind="ExternalInput")
    wv = nc.dram_tensor("wv", [D, JC], F8, kind="ExternalInput")
    wv_bf = nc.dram_tensor("wv_bf", [D, JC], BF, kind="ExternalInput")
    wo = nc.dram_tensor("wo", [JC, D], BF, kind="ExternalInput")
    bq = nc.dram_tensor("bq", [JC, 1], FP, kind="ExternalInput")
    bk = nc.dram_tensor("bk", [JC, 1], FP, kind="ExternalInput")
    y = nc.dram_tensor("y", [T, D], BF, kind="ExternalOutput")
    # schedule-tuning knobs (env for experiments)
    # exp offload: "s1,s2,s3" = for i-blocks 1..3, offload head-1's half of
    # every s-th full chunk to DVE (0 = none)
    exp_off = [0] + [int(v) for v in os.environ.get("K_EXP_OFF", "0,0,0").split(",")]
    yact = os.environ.get("K_YACT", "1") == "1"
    yct = int(os.environ.get("K_YCT", "12"))
    trpact = os.environ.get("K_TRPACT", "0") == "1"
    ystag = os.environ.get("K_YSTAG", "1") == "1"
    n_warm = int(os.environ.get("K_WARM", "56"))
    w_full = float(os.environ.get("K_W_FULL", "3.0"))
    w_diag = float(os.environ.get("K_W_DIAG", "1.0"))
    w0ramp = float(os.environ.get("K_W0RAMP", "0"))  # 0 = uniform block-0 weights

    with tile.TileContext(nc) as tc:
        import contextlib

        with contextlib.ExitStack() as ctx:
            singles = ctx.enter_context(tc.tile_pool(name="singles", bufs=1))
            xt_pool = ctx.enter_context(tc.tile_pool(name="xt", bufs=2))
            # all of an i-block pair's exp tiles stay live until its AV pass
            exp_pool = ctx.enter_context(tc.tile_pool(name="exp", bufs=32))
            nrm_pool = ctx.enter_context(tc.tile_pool(name="nrm", bufs=4))
            ysb_pool = ctx.enter_context(tc.tile_pool(name="ysb", bufs=4))
            # PSUM: tag "big" 2x[128,1024] f32 (qT/kT proj pairs, then score
            # pairs), "av" 2x[128,260] f32 (AV + denominators, one per head),
            # "fil" 2x[128,512] (proj, aoT transposes, out-proj) = 8 banks.
            ps = ctx.enter_context(tc.tile_pool(name="ps", bufs=2, space="PSUM"))

            def load_block(tb, halves=1):
                # fp8 xT always; bf16 xT only for block 0 (bf16 v-projection)
                xt8 = xt_pool.tile([P, NDC * TS], F8, tag="xt8", name=f"xt8_{tb}")
                src8 = xt8_d[:, :].rearrange("(c p) t -> p c t", p=P)[
                    :, :, TS * tb:TS * (tb + 1)]
                h = NDC // halves
                for k in range(halves):
                    nc.sync.dma_start(
                        out=xt8.rearrange("p (c t) -> p c t", t=TS)[
                            :, h * k:h * (k + 1), :],
                        in_=src8[:, h * k:h * (k + 1), :],
                    )
                xt = None
                if tb == 0:
                    xt = xt_pool.tile([P, NDC * TS], BF, tag="xt", name=f"xt{tb}")
                    src = xt_d[:, :].rearrange("(c p) t -> p c t", p=P)
                    for k in range(halves):
                        nc.sync.dma_start(
                            out=xt.rearrange("p (c t) -> p c t", t=TS)[
                                :, h * k:h * (k + 1), :],
                            in_=src[:, h * k:h * (k + 1), :],
                        )
                return xt, xt8

            # ---- DMA order = need order: wq/bq + first xt half unblock the
            # first projection ~2.5us in; wo is not needed until ~40us ----
            wq_sb = singles.tile([P, NDC * JC], F8, tag="wq")  # chunk c at [JC*c, JC*(c+1))
            wk_sb = singles.tile([P, NDC * JC], F8, tag="wk")
            wv_sb = singles.tile([P, NDC * JC], F8, tag="wv")
            wvb_sb = singles.tile([P, NDC * JC], BF, tag="wvb")
            bq_sb = singles.tile([P, 2], FP, tag="bq")
            bk_sb = singles.tile([P, 2], FP, tag="bk")
            wo_sb = singles.tile([P, 2 * D], BF, tag="wo")      # j-chunk jc at [D*jc, ...)
            nc.sync.dma_start(
                out=wq_sb.rearrange("p (c j) -> p c j", j=JC),
                in_=wq[:, :].rearrange("(c p) j -> p c j", p=P),
            )
            nc.sync.dma_start(
                out=bq_sb, in_=bq[:, :].rearrange("(j p) one -> p (j one)", p=P),
            )
            nc.sync.dma_start(
                out=wk_sb.rearrange("p (c j) -> p c j", j=JC),
                in_=wk[:, :].rearrange("(c p) j -> p c j", p=P),
            )
            nc.sync.dma_start(
                out=bk_sb, in_=bk[:, :].rearrange("(j p) one -> p (j one)", p=P),
            )
            xt0 = load_block(0, halves=2)
            nc.sync.dma_start(
                out=wvb_sb.rearrange("p (c j) -> p c j", j=JC),
                in_=wv_bf[:, :].rearrange("(c p) j -> p c j", p=P),
            )
            nc.sync.dma_start(
                out=wv_sb.rearrange("p (c j) -> p c j", j=JC),
                in_=wv[:, :].rearrange("(c p) j -> p c j", p=P),
            )
            nc.sync.dma_start(
                out=wo_sb.rearrange("p (c d) -> p c d", d=D),
                in_=wo[:, :].rearrange("(c p) d -> p c d", p=P),
            )

            # bf16 identity for the ao PE transposes
            ident = singles.tile([P, P], BF)
            nc.vector.memset(ident, 0.0)
            nc.gpsimd.affine_select(
                out=ident, in_=ident, compare_op=mybir.AluOpType.not_equal,
                fill=1.0, base=0, channel_multiplier=1, pattern=[[-1, P]],
            )
            # strictly-upper-triangular -BIG tile (transposed; stationary of
            # the causal-mask matmul: TRIT.T @ I adds -BIG where key > query)
            trit = singles.tile([P, P], BF)
            nc.vector.memset(trit, 0.0)
            nc.gpsimd.affine_select(
                out=trit, in_=trit, compare_op=mybir.AluOpType.is_ge,
                fill=-60.0, base=0, channel_multiplier=1, pattern=[[-1, P]],
            )

            # persistent activations
            qt_sb = [singles.tile([P, T], BF, tag=f"qt{j}", name=f"qt_sb{j}") for j in range(2)]
            kt_sb = [singles.tile([P, T], BF, tag=f"kt{j}", name=f"kt_sb{j}") for j in range(2)]
            # attention out, natural layout: tile tt holds [128 t, 4 heads x 64]
            ao_nat = singles.tile([P, (T // P) * HL * DH], BF, tag="aon")
            # attention out transposed: j-chunk jc at cols [T*jc, T*(jc+1))
            aoT_sb = singles.tile([P, 2 * T], BF, tag="aot")
            # v_aug fp8 (all chunks): l-chunk lc at [VA*HL*lc, ...), head h at
            # offset VA*h, ones at +DH.  bf16 copy of chunks 0-3 for block-0 AV.
            n_lch = T // LCH
            vaug = singles.tile([P, n_lch * HL * VA], F8, tag="vaug")
            vaug_g = vaug.rearrange("p (c v) -> p c v", v=VA)
            vaug_p = vaug.rearrange("p (c h v) -> p c h v", h=HL, v=VA)
            nc.vector.memset(vaug_g[:, :, DH], 1.0)   # ones columns
            vaugb = singles.tile([P, NIT * HL * VA], BF, tag="vaugb")
            vaugb_g = vaugb.rearrange("p (c v) -> p c v", v=VA)
            nc.vector.memset(vaugb_g[:, :, DH], 1.0)

            # PE warm-up: dummy matmuls during the initial DMA wait get the
            # clock to full rate before the real work arrives.
            warm = ps.tile([P, P], FP, tag="fil", name="warm")
            junk = singles.tile([P, P], BF, tag="junk")
            nc.vector.memset(junk, 0.0)
            for _ in range(8):
                nc.tensor.matmul(warm, junk, junk, start=True, stop=True)
            for _ in range(n_warm - 8):
                nc.tensor.matmul(warm, ident, ident, start=True, stop=True)

            # ---------- emission units (software-pipelined schedule) ----------
            def proj_units(tb, xt, xt8, split_v=False):
                """Single-bank filler units: q/k transposed per j-tile, v in
                natural [token, head-col] layout straight into v_aug."""
                units = []
                xt8_g = xt8.rearrange("p (c t) -> p c t", t=TS)

                def qk_mm(acc, w_sb, j, c, start, stop):
                    # fp8 DoubleRow: chunk PAIR (2c, 2c+1) packed as k-tiles
                    w_g = w_sb.rearrange("p (c j) -> p c j", j=JC)
                    nc.tensor.matmul(
                        acc,
                        w_g[:, 2 * c:2 * c + 2, P * j:P * (j + 1)],
                        xt8_g[:, 2 * c:2 * c + 2, :],
                        start=start, stop=stop,
                        perf_mode=mybir.MatmulPerfMode.DoubleRow,
                    )

                def make_qk(which, w_sb, out_sb, j):
                    box = [None]
                    nqk = NDC // 2   # chunk-pairs

                    def emit_lo():
                        box[0] = ps.tile([P, TS], FP, tag="fil", name=f"{which}p{tb}_{j}")
                        for c in range(nqk // 2):
                            qk_mm(box[0], w_sb, j, c, start=(c == 0), stop=False)

                    def emit_hi():
                        acc = box[0]
                        for c in range(nqk // 2, nqk):
                            qk_mm(acc, w_sb, j, c, start=False, stop=(c == nqk - 1))
                        if which == "qt":
                            nc.vector.tensor_scalar(
                                out=out_sb[j][:, TS * tb:TS * (tb + 1)], in0=acc,
                                scalar1=0.125, scalar2=bq_sb[:, j:j + 1],
                                op0=mybir.AluOpType.mult, op1=mybir.AluOpType.add,
                            )
                        else:
                            nc.vector.tensor_scalar(
                                out=out_sb[j][:, TS * tb:TS * (tb + 1)], in0=acc,
                                scalar1=bk_sb[:, j:j + 1], scalar2=None,
                                op0=mybir.AluOpType.add,
                            )
                    return [emit_lo, emit_hi]

                def make_v(s):
                    box = [None]

                    def make_w(w):
                        def emit():
                            # natural [t, j] layout (xT chunk is the stationary);
                            # sequential accumulation groups per bank half
                            if w == 0:
                                box[0] = ps.tile([P, TS], FP, tag="fil", name=f"vp{tb}_{s}")
                            acc = box[0]
                            ts_ = 2 * s + w
                            if tb == 0:
                                # bf16: block-0 queries need full-precision v
                                for c in range(NDC):
                                    nc.tensor.matmul(
                                        acc[:, JC * w:JC * (w + 1)],
                                        xt[:, TS * c + P * ts_:TS * c + P * (ts_ + 1)],
                                        wvb_sb[:, JC * c:JC * (c + 1)],
                                        start=(c == 0), stop=(c == NDC - 1),
                                    )
                            else:
                                # fp8 DoubleRow over chunk pairs
                                wv_g = wv_sb.rearrange("p (c j) -> p c j", j=JC)
                                for c in range(NDC // 2):
                                    nc.tensor.matmul(
                                        acc[:, JC * w:JC * (w + 1)],
                                        xt8_g[:, 2 * c:2 * c + 2,
                                              P * ts_:P * (ts_ + 1)],
                                        wv_g[:, 2 * c:2 * c + 2, :],
                                        start=(c == 0), stop=(c == NDC // 2 - 1),
                                        perf_mode=mybir.MatmulPerfMode.DoubleRow,
                                    )
                            lc = 4 * tb + ts_
                            if tb == 0:
                                # bf16 copy for block-0 AV; fp8 vaug derived
                                # SBUF->SBUF on gpsimd (no PSUM port there)
                                nc.vector.tensor_copy(
                                    out=vaugb_g[:, HL * lc:HL * (lc + 1), 0:DH],
                                    in_=acc[:, JC * w:JC * (w + 1)].rearrange(
                                        "p (h d) -> p h d", d=DH
                                    ),
                                )
                                nc.gpsimd.tensor_copy(
                                    out=vaug_g[:, HL * lc:HL * (lc + 1), 0:DH],
                                    in_=vaugb_g[:, HL * lc:HL * (lc + 1), 0:DH],
                                )
                            else:
                                nc.vector.tensor_copy(
                                    out=vaug_g[:, HL * lc:HL * (lc + 1), 0:DH],
                                    in_=acc[:, JC * w:JC * (w + 1)].rearrange(
                                        "p (h d) -> p h d", d=DH
                                    ),
                                )
                        return emit
                    return [make_w(0), make_w(1)]

                qk_by_j = [[], []]
                for j in range(2):
                    qk_by_j[j].extend(make_qk("qt", wq_sb, qt_sb, j))
                    qk_by_j[j].extend(make_qk("kt", wk_sb, kt_sb, j))
                v_units = []
                for s in range(2):
                    v_units.extend(make_v(s))
                if split_v:
                    return qk_by_j[0], qk_by_j[1], v_units
                return qk_by_j[0] + qk_by_j[1] + v_units

            def attn_units(i, inject_v=None):
                """Returns (close_units, close_weights): full-chunk stretch is
                ACT(exp)-paced (weight ~ starvation); close stretch (diagonal
                chunks + AV windows + tails) is PE-rich.

                inject_v (block 0): the block's own v-projection units, placed
                right after each diagonal chunk so they're off the first-score
                critical path but still precede the AV window that reads their
                vaugb chunk."""
                nch = 4 * (i + 1)   # causal chunks
                fp8 = i >= 1        # hybrid: block 0 keeps bf16 AV
                cunits = []
                cweights = []
                for jp in range(2):          # head pair (2*jp, 2*jp+1)
                    avs = [None, None]
                    # i==0: per-chunk bf16 tiles; i>=1: fp8 pair tiles
                    exs = [None] * nch
                    ex2s = [None] * ((nch + 1) // 2)

                    def make_pair_start(i, jp, avs):
                        def emit():
                            for u in range(2):
                                avs[u] = ps.tile(
                                    [P, NIT * VA], FP, tag="av", name=f"av{i}_{2 * jp + u}"
                                )
                        return emit

                    def make_c(i, jp, exs, ex2s, c, dve_exp=False):
                        def emit():
                            # Diagonal chunks only need query columns >= 128*v
                            diag = c >= 4 * i
                            v = c - 4 * i if diag else 0
                            off = P * v
                            # both heads' scoresT for chunk c in one 2-bank tile
                            sc = ps.tile([P, 2 * IB], FP, tag="big", name=f"sc{i}_{jp}_{c}")
                            for u in range(2):
                                ro = DH * u
                                nc.tensor.matmul(
                                    sc[:, IB * u + off:IB * (u + 1)],
                                    kt_sb[jp][ro:ro + DH, LCH * c:LCH * (c + 1)],
                                    qt_sb[jp][ro:ro + DH, IB * i + off:IB * (i + 1)],
                                    start=True, stop=not diag,
                                )
                                if diag:
                                    # causal mask: add -60 where key > query
                                    nc.tensor.matmul(
                                        sc[:, IB * u + off:IB * u + off + P],
                                        trit, ident,
                                        start=False, stop=True,
                                        skip_group_check=True,
                                    )
                            sc_g = sc.rearrange("p (u n) -> p u n", u=2)
                            if not fp8:
                                ex = exp_pool.tile([P, 2 * IB], BF, tag="ex",
                                                   name=f"ex{i}_{jp}_{c}")
                                exs[c] = ex
                                ex_t = ex.rearrange("p (u n) -> p u n", u=2)[:, :, off:]
                            else:
                                if c % 2 == 0:
                                    ex2s[c // 2] = exp_pool.tile(
                                        [P, 2 * 2 * IB], F8, tag="ex",
                                        name=f"ex{i}_{jp}_{c // 2}p")
                                ex2 = ex2s[c // 2]
                                ex_t = ex2.rearrange(
                                    "p (k u n) -> p k u n", k=2, u=2
                                )[:, c % 2, :, off:]
                            if dve_exp:
                                # split the chunk's exp across engines: ACT
                                # does head 0; DVE does head 1 via the bf16
                                # bit-trick (bits of e^x ~ round(x*128/ln2 +
                                # magic)); idle Pool converts bf16->fp8
                                # SBUF->SBUF (gpsimd has no PSUM port and
                                # int8 DVE writes crash)
                                ex_th = ex2.rearrange(
                                    "p (k u n) -> p k u n", k=2, u=2)
                                nc.scalar.activation(
                                    out=ex_th[:, c % 2, 0, off:],
                                    in_=sc_g[:, 0, off:],
                                    func=mybir.ActivationFunctionType.Exp,
                                )
                                scr = nrm_pool.tile([P, IB], BF, tag="exs",
                                                    name=f"exs{i}_{jp}_{c}")
                                nc.vector.tensor_scalar(
                                    out=scr.bitcast(mybir.dt.int16),
                                    in0=sc_g[:, 1, :],
                                    scalar1=184.66500816464, scalar2=16248.6,
                                    op0=mybir.AluOpType.mult,
                                    op1=mybir.AluOpType.add,
                                )
                                nc.gpsimd.tensor_copy(
                                    out=ex_th[:, c % 2, 1, :], in_=scr,
                                )
                            else:
                                nc.scalar.activation(
                                    out=ex_t, in_=sc_g[:, :, off:],
                                    func=mybir.ActivationFunctionType.Exp,
                                )
                        return emit

                    def make_av(i, jp, avs, exs, ex2s, u, it):
                        # one contiguous PSUM accumulation window per (head,
                        # i-tile)
                        def emit():
                            h = 2 * jp + u
                            cnt = 4 * i + it + 1
                            out = avs[u][:, VA * it:VA * (it + 1)]
                            if not fp8:
                                for c in range(cnt):
                                    nc.tensor.matmul(
                                        out,
                                        exs[c][:, IB * u + P * it:IB * u + P * (it + 1)],
                                        vaugb_g[:, HL * c + h, :],
                                        start=(c == 0), stop=(c == cnt - 1),
                                        skip_group_check=True,
                                    )
                            else:
                                npair = cnt // 2
                                odd = cnt % 2
                                for pc in range(npair):
                                    ex_pair = ex2s[pc].rearrange(
                                        "p (k u n) -> p k u n", k=2, u=2
                                    )[:, :, u, P * it:P * (it + 1)]
                                    nc.tensor.matmul(
                                        out,
                                        ex_pair,
                                        vaug_p[:, 2 * pc:2 * pc + 2, h, :],
                                        start=(pc == 0), stop=(not odd and pc == npair - 1),
                                        perf_mode=mybir.MatmulPerfMode.DoubleRow,
                                        skip_group_check=True,
                                    )
                                if odd:
                                    ex_last = ex2s[npair].rearrange(
                                        "p (k u n) -> p k u n", k=2, u=2
                                    )[:, 0, u, P * it:P * (it + 1)]
                                    nc.tensor.matmul(
                                        out,
                                        ex_last,
                                        vaug_g[:, HL * (cnt - 1) + h, :],
                                        start=(npair == 0), stop=True,
                                        skip_group_check=True,
                                    )
                        return emit

                    def make_tail(i, jp, avs, u):
                        def emit():
                            h = 2 * jp + u
                            av_g = avs[u].rearrange("p (t v) -> p t v", v=VA)
                            rc = nrm_pool.tile([P, NIT], FP, tag="rc", name=f"rc{i}_{h}")
                            nc.vector.reciprocal(out=rc, in_=av_g[:, :, DH])
                            # one broadcast multiply normalizes all 4 i-tiles
                            ao_v = ao_nat.rearrange("p (t j) -> p t j", j=JC)[
                                :, NIT * i:NIT * (i + 1),
                                P * jp + DH * u:P * jp + DH * (u + 1)]
                            rc_b = rc.rearrange("p (t o) -> p t o", o=1).to_broadcast(
                                [P, NIT, DH])
                            nc.vector.tensor_tensor(
                                out=ao_v, in0=av_g[:, :, 0:DH], in1=rc_b,
                                op=mybir.AluOpType.mult,
                            )
                        return emit

                    units, weights = cunits, cweights   # per-jp: full then close
                    for c in range(4 * i):
                        s = exp_off[i]
                        dve = s > 0 and c % s == s - 1
                        units.append(make_c(i, jp, exs, ex2s, c, dve_exp=dve))
                        weights.append(w_full)
                    cunits.append(make_pair_start(i, jp, avs))
                    cweights.append(0.0)
                    # AV window (u, it) completes with diagonal chunk 4i+it:
                    # emit it one diag chunk later so its tail never waits on
                    # the freshest exp
                    for v in range(NIT):
                        cunits.append(make_c(i, jp, exs, ex2s, 4 * i + v))
                        cweights.append(w_diag * (w0ramp * (v + 1) / NIT
                                                  if i == 0 and w0ramp else 1.0))
                        if inject_v is not None and jp == 0:
                            # vaugb chunk v must land before AV window v reads
                            # it (window v emitted after diag chunk v+1)
                            cunits.append(inject_v[v])
                            cweights.append(w_diag)
                        if v >= 1:
                            for u in range(2):
                                cunits.append(make_av(i, jp, avs, exs, ex2s, u, v - 1))
                                cweights.append(0.0)
                    for u in range(2):
                        cunits.append(make_av(i, jp, avs, exs, ex2s, u, NIT - 1))
                        cweights.append(0.0)
                    cunits.append(make_tail(i, jp, avs, 0))
                    cweights.append(0.0)
                    cunits.append(make_tail(i, jp, avs, 1))
                    cweights.append(0.0)
                return cunits, cweights

            def y_units(i):
                units = []

                def make(tt):
                    ysb_box = [None]

                    def emit_tr():
                        trp = ps.tile([P, 2 * P], BF, tag="fil", name=f"tr{tt}")
                        for jc in range(2):
                            nc.tensor.transpose(
                                trp[:, P * jc:P * (jc + 1)],
                                ao_nat[:, JC * tt + P * jc:JC * tt + P * (jc + 1)],
                                ident,
                            )
                        # one copy moves both j-chunks into aoT (jc-major);
                        # tail-region copies alternate onto ACT (idle after
                        # the final exps)
                        dst = aoT_sb.rearrange("p (j t) -> p j t", j=2)[
                            :, :, P * tt:P * (tt + 1)]
                        src = trp.rearrange("p (j t) -> p j t", j=2)
                        if trpact and tt >= yct and tt % 2 == 0:
                            nc.scalar.activation(
                                out=dst, in_=src,
                                func=mybir.ActivationFunctionType.Copy,
                            )
                        else:
                            nc.vector.tensor_copy(out=dst, in_=src)

                    def make_yp(db):
                        def emit():
                            if db == 0:
                                ysb_box[0] = ysb_pool.tile(
                                    [P, D], BF, tag="ysb", name=f"ysb{tt}"
                                )
                            ysb = ysb_box[0]
                            yps = ps.tile([P, IB], FP, tag="fil", name=f"yps{tt}_{db}")
                            for jc in range(2):
                                nc.tensor.matmul(
                                    yps,
                                    aoT_sb[:, T * jc + P * tt:P * (tt + 1) + T * jc],
                                    wo_sb[:, D * jc + IB * db:D * jc + IB * (db + 1)],
                                    start=(jc == 0), stop=(jc == 1),
                                )
                            # y copies on DVE; optionally the last block's go
                            # to ACT (idle once the final exps are done)
                            if yact and tt >= yct and (tt + db) % 2 == 0:
                                nc.scalar.activation(
                                    out=ysb[:, IB * db:IB * (db + 1)], in_=yps,
                                    func=mybir.ActivationFunctionType.Copy,
                                )
                            else:
                                nc.vector.tensor_copy(
                                    out=ysb[:, IB * db:IB * (db + 1)], in_=yps,
                                )
                            nc.sync.dma_start(
                                out=y[P * tt:P * (tt + 1), IB * db:IB * (db + 1)],
                                in_=ysb[:, IB * db:IB * (db + 1)],
                            )
                        return emit
                    return [emit_tr, make_yp(0), make_yp(1)]

                per_tt = [make(tt) for tt in range(NIT * i, NIT * (i + 1))]
                if ystag:
                    # staggered: tr(tt+1) runs while tt's aoT copy drains, so
                    # PE never stalls on the DVE copy between transposes and
                    # the out-proj matmuls
                    order = [(0, 0), (1, 0), (0, 1), (0, 2), (2, 0), (1, 1),
                             (1, 2), (3, 0), (2, 1), (2, 2), (3, 1), (3, 2)]
                    units = [per_tt[a][b] for a, b in order]
                else:
                    units = [u for tt_units in per_tt for u in tt_units]
                return units

            def interleave(main, fillers, weights=None):
                """Emit `main` units with `fillers` spread between them,
                proportionally to per-unit `weights` (PE-starvation demand)."""
                if not main:
                    for f in fillers:
                        f()
                    return
                if weights is None:
                    weights = [1.0] * len(main)
                tot = sum(weights) or 1.0
                nf = len(fillers)
                fi = 0
                acc = 0.0
                for m, w in zip(main, weights):
                    m()
                    acc += w
                    want = int(round(acc / tot * nf))
                    while fi < want:
                        fillers[fi]()
                        fi += 1
                while fi < nf:
                    fillers[fi]()
                    fi += 1

            def merge(ua, ub):
                """Proportional round-robin merge of two unit streams."""
                out_u = []
                ia = ib2 = 0
                na, nb = len(ua), len(ub)
                for _ in range(na + nb):
                    if ib2 >= nb or (ia < na and ia * nb <= ib2 * na):
                        out_u.append(ua[ia]); ia += 1
                    else:
                        out_u.append(ub[ib2]); ib2 += 1
                return out_u

            # ---------- pipelined schedule ----------
            # NOTE: Tile is a *tracing* scheduler — emission order defines the
            # dataflow. Every consumer must be emitted after its producer.
            # Prologue: only block 0's j-tile-0 q/k projections (they gate
            # the first scores); j1's ride as attn0's first fillers and the
            # v units are injected into attn0 after the diag chunks, so the
            # first exp doesn't wait behind them on the in-order PE.
            headmode = int(os.environ.get("K_HEADMODE", "0"))
            qk0_j0, qk0_j1, v0 = proj_units(0, *xt0, split_v=True)
            for u in qk0_j0:
                u()
            if headmode < 2:
                for u in qk0_j1:
                    u()
            if headmode == 0:
                for u in v0:
                    u()
                v0 = None

            # block i's attention interleaves the next block's load +
            # projections AND (yspread) block i-1's out-projections (ao rows
            # final after block i-1's tails) — pulls y work off the tail
            yspread = int(os.environ.get("K_YSPREAD", "0"))
            for tb in range(3):
                cu, cw = attn_units(tb, inject_v=v0 if tb == 0 else None)
                nxt = load_block(tb + 1)
                fillers = proj_units(tb + 1, *nxt)
                if tb == 0 and headmode >= 2:
                    fillers = qk0_j1 + fillers
                if tb >= 1 and yspread:
                    yu = y_units(tb - 1)
                    fillers = merge(fillers, yu) if yspread == 1 else fillers + yu
                interleave(cu, fillers, cw)
            cu, cw = attn_units(3)
            rest = y_units(2) if yspread else y_units(0) + y_units(1) + y_units(2)
            interleave(cu, rest, cw)
            for u in y_units(NTB - 1):
                u()

    nc.compile()
    return nc


def get_nc():
    if "nc" not in _CACHE:
        _CACHE["nc"] = build_nc()
    return _CACHE["nc"]


def kernel(x, wq, bq, wk, bk, wv, bv, wo, bo):
    import ml_dtypes
    BF_NP = ml_dtypes.bfloat16
    F8_NP = ml_dtypes.float8_e4m3

    x = np.asarray(x, dtype=np.float32)
    wq = np.asarray(wq, dtype=np.float32)
    wk = np.asarray(wk, dtype=np.float32)
    wv = np.asarray(wv, dtype=np.float32)
    wo = np.asarray(wo, dtype=np.float32)
    bq = np.asarray(bq, dtype=np.float32)
    bk = np.asarray(bk, dtype=np.float32)
    bv = np.asarray(bv, dtype=np.float32)
    bo = np.asarray(bo, dtype=np.float32)

    nc = get_nc()
    xt_f32 = [np.ascontiguousarray(x[b].T) for b in range(B)]
    xt_by_batch = [np.ascontiguousarray(xb[:, :TS]).astype(BF_NP) for xb in xt_f32]
    xt8_by_batch = [xb.astype(F8_NP) for xb in xt_f32]
    wq_c = wq.astype(F8_NP)
    wk_c = wk.astype(F8_NP)
    wv_f8 = wv.astype(F8_NP)
    wv_bf = wv.astype(BF_NP)
    wo_bf = wo.astype(BF_NP)

    in_maps = []
    for core in range(NCORES):
        b, g = divmod(core, GROUPS)
        cs = slice(JC * g, JC * (g + 1))
        im = {
            "xt": xt_by_batch[b],
            "xt8": xt8_by_batch[b],
            "wq": np.ascontiguousarray(wq_c[:, cs]),
            "wk": np.ascontiguousarray(wk_c[:, cs]),
            "wv": np.ascontiguousarray(wv_f8[:, cs]),
            "wv_bf": np.ascontiguousarray(wv_bf[:, cs]),
            "wo": np.ascontiguousarray(wo_bf[cs, :]),
            "bq": np.ascontiguousarray(bq[cs].reshape(JC, 1)),
            "bk": np.ascontiguousarray(bk[cs].reshape(JC, 1)),
        }
        in_maps.append(im)
    res = run_bass_kernel_spmd(nc, in_maps, list(range(NCORES)))
    _CACHE["last_results"] = res

    out = np.zeros((B, S, D), np.float32)
    for core in range(NCORES):
        out[core // GROUPS] += res.results[core]["y"]
    # bv and bo never pass through softmax nonlinearity: rows of attn sum to 1,
    # so (v + bv) contributes exactly bv @ wo to every output row.
    out += (bv @ wo + bo)[None, None, :]
    return out
